# revision 1
# baseline (speedup 1.0000x reference)
"""Trainium2 Bass kernel for nn_GPTQOFTLinear.

y = (x rotated by block-diagonal Cayley(oft_r)) @ W^T + b

Strategy (8 NeuronCores, no collectives):
  - Data-parallel shard x over the 8192 tokens (1024 tokens/core); W, oft_r, b
    replicated.
  - On each core:
      1. Cayley transform Q_b = (I - S_b)(I + S_b)^{-1} for the 64 diagonal
         64x64 blocks, computed as pairs packed into 32 block-diagonal 128x128
         matrices.  Uses the commuting/symmetric form
         Q = (I - S)^2 (I - S^2)^{-1} with Newton iteration for the inverse
         (all iterates symmetric -> no transposes needed on device).
      2. Rotate: x_rot^T[:, j-tile] = Q_pair^T-free matmuls into an
         SBUF-resident x_rot^T [128, 32, 1024].
      3. Main matmul y[t, o] = sum_j x_rot^T[j, t] * W^T[j, o] + b[o],
         streaming W^T from HBM, accumulating in PSUM over 32 k-tiles.
  - Host side does only layout (shard/transpose/zero-pad/replicate), no math.
"""

import os
import sys

for _p in ("/opt/trn_rl_repo",):
    if _p not in sys.path and os.path.isdir(_p):
        sys.path.append(_p)

import numpy as np

import concourse.bass as bass  # noqa: E402
import concourse.mybir as mybir  # noqa: E402
import concourse.tile as tile  # noqa: E402
from concourse import bacc  # noqa: E402
from concourse.bass_utils import run_bass_kernel_spmd  # noqa: E402

# Problem shapes (hardcoded per contract).
BATCH, SEQ = 2, 4096
DIN = 4096
DOUT = 4096
BS = 64                      # oft block size
RANK = DIN // BS             # 64 blocks
N_CORES = 8
TOK = BATCH * SEQ            # 8192 tokens
TPC = TOK // N_CORES         # 1024 tokens per core
P = 128
JT = DIN // P                # 32 contraction tiles
NPAIR = RANK // 2            # 32 block pairs
NT = TPC // P                # 8 token tiles per core
OGW = 512                    # output-feature group width
OG = DOUT // OGW             # 8 output groups
NEWTON_ITERS = 3

F32 = mybir.dt.float32

# fp32r streams fp32 data through the PE at full (1 cycle/row) rate with
# slightly reduced multiply precision; plain fp32 is exact but 4 cycles/row.
_MAIN_DT = {"fp32": F32, "fp32r": mybir.dt.float32r}[os.environ.get("KERNEL_MAIN_DT", "fp32r")]
_ROT_DT = {"fp32": F32, "fp32r": mybir.dt.float32r}[os.environ.get("KERNEL_ROT_DT", "fp32r")]

_CACHE: dict = {}


def _emit(nc, tc, xT, wT, G, Gt, eye, bias_rep, y):
    """Emit the whole per-core program under TileContext tc."""
    from contextlib import ExitStack

    ctx = ExitStack()
    with ctx:
        # ---- persistent pools (allocated first, stable addresses) ----
        const = ctx.enter_context(tc.tile_pool(name="const", bufs=1))
        xrot_pool = ctx.enter_context(tc.tile_pool(name="xrotp", bufs=1))
        wt_pool = ctx.enter_context(tc.tile_pool(name="wtp", bufs=3))
        out_pool = ctx.enter_context(tc.tile_pool(name="outp", bufs=4))
        bias_pool = ctx.enter_context(tc.tile_pool(name="biasp", bufs=2))
        xstage_pool = ctx.enter_context(tc.tile_pool(name="xstagep", bufs=2))

        eye_sb = const.tile([P, P], F32, name="eye_sb", tag="eye")
        nc.sync.dma_start(out=eye_sb, in_=eye)

        # fp32r operands must be *produced* as fp32r (walrus verifier tracks
        # the rounding through dataflow), so the tiles feeding the fast
        # matmuls are declared fp32r rather than bitcast at the call site.
        xrot = xrot_pool.tile([P, JT, TPC], _MAIN_DT, name="xrot", tag="xrot")

        # ---- Cayley + rotation scope (its PSUM/SBUF freed before main) ----
        with tc.tile_pool(name="qpool", bufs=1) as qpool, \
             tc.tile_pool(name="cay", bufs=2) as cay, \
             tc.tile_pool(name="cpsum", bufs=2, space="PSUM") as cpsum, \
             tc.tile_pool(name="rpsum", bufs=2, space="PSUM") as rpsum:

            Q = qpool.tile([P, NPAIR, P], _ROT_DT, name="Q", tag="Q")

            for p in range(NPAIR):
                def ct(tagname):
                    return cay.tile([P, P], F32, name=tagname, tag=tagname)

                g = ct("cay_g")
                nc.sync.dma_start(out=g, in_=G[:, p, :])
                gt = ct("cay_gt")
                nc.sync.dma_start(out=gt, in_=Gt[:, p, :])

                s2 = ct("cay_s2")           # 2S
                nc.vector.tensor_sub(s2, g, gt)
                ns2 = ct("cay_ns2")         # -2S
                nc.vector.tensor_sub(ns2, gt, g)

                # C = S @ S = 0.25 * (s2^T)^T @ s2, lhsT = s2^T = ns2
                ps = cpsum.tile([P, P], F32, name="cps", tag="cps")
                nc.tensor.matmul(ps, ns2, s2)
                c = ct("cay_c")
                nc.vector.tensor_scalar_mul(c, ps, 0.25)

                e = ct("cay_e")             # E = I - C (symmetric)
                nc.vector.tensor_sub(e, eye_sb, c)
                x_new = ct("cay_x0")        # X0 = I + C
                nc.vector.tensor_add(x_new, eye_sb, c)

                for it in range(NEWTON_ITERS):
                    ps_t = cpsum.tile([P, P], F32, name="cps", tag="cps")
                    nc.tensor.matmul(ps_t, e, x_new)          # T = E @ X
                    u = ct(f"cay_u{it}")                       # U = 2I - T
                    nc.vector.scalar_tensor_tensor(
                        u, eye_sb, 2.0, ps_t,
                        mybir.AluOpType.mult, mybir.AluOpType.subtract,
                    )
                    ps_x = cpsum.tile([P, P], F32, name="cps", tag="cps")
                    nc.tensor.matmul(ps_x, x_new, u)          # X' = X @ U
                    x_new = ct(f"cay_x{it + 1}")
                    nc.vector.tensor_copy(out=x_new, in_=ps_x)

                # Ft = F^T = I + 2S + C  (F = (I-S)^2 = I - 2S + C)
                ftt = ct("cay_ftt")
                nc.vector.tensor_add(ftt, s2, c)
                ft = ct("cay_ft")
                nc.vector.tensor_add(ft, ftt, eye_sb)

                # Q_pair = F @ D = Ft^T @ X
                ps_q = cpsum.tile([P, P], F32, name="cps", tag="cps")
                nc.tensor.matmul(ps_q, ft, x_new)
                nc.vector.tensor_copy(out=Q[:, p, :], in_=ps_q)

            # ---- rotation: x_rot^T[:, j, :] = Q_j^T.T @ x^T[j-tile] ----
            for j in range(JT):
                xs = xstage_pool.tile([P, TPC], _ROT_DT, name="xs", tag="xs")
                nc.sync.dma_start(out=xs, in_=xT[j * P:(j + 1) * P, :])
                for th in range(TPC // 512):
                    rps = rpsum.tile([P, 512], F32, name="rps", tag="rps")
                    nc.tensor.matmul(
                        rps,
                        Q[:, j, :],
                        xs[:, th * 512:(th + 1) * 512],
                    )
                    nc.vector.tensor_copy(
                        out=xrot[:, j, th * 512:(th + 1) * 512], in_=rps)

        # ---- main matmul (all 8 PSUM banks) ----
        with tc.tile_pool(name="mpsum", bufs=1, space="PSUM") as mpsum:
            for og in range(OG):
                bias_og = bias_pool.tile([P, OGW], F32, name="bias_og", tag="bias_og")
                nc.sync.dma_start(out=bias_og, in_=bias_rep[:, og * OGW:(og + 1) * OGW])

                psums = [
                    mpsum.tile([P, OGW], F32, name=f"mps{tt}", tag=f"mps{tt}")
                    for tt in range(NT)
                ]
                for j in range(JT):
                    wt = wt_pool.tile([P, OGW], _MAIN_DT, name="wt", tag="wt")
                    nc.sync.dma_start(
                        out=wt,
                        in_=wT[j * P:(j + 1) * P, og * OGW:(og + 1) * OGW])
                    for tt in range(NT):
                        nc.tensor.matmul(
                            psums[tt],
                            xrot[:, j, tt * P:(tt + 1) * P],
                            wt[:],
                            start=(j == 0),
                            stop=(j == JT - 1),
                        )
                for tt in range(NT):
                    out_sb = out_pool.tile([P, OGW], F32, name="out_sb", tag="out_sb")
                    nc.vector.tensor_add(out_sb, psums[tt], bias_og)
                    nc.sync.dma_start(
                        out=y[tt * P:(tt + 1) * P, og * OGW:(og + 1) * OGW],
                        in_=out_sb)


def _build():
    key = (_MAIN_DT, _ROT_DT)
    if key in _CACHE:
        return _CACHE[key]
    nc = bacc.Bacc("TRN2", target_bir_lowering=False, debug=False,
                   num_devices=N_CORES)
    xT = nc.dram_tensor("xT", [DIN, TPC], _ROT_DT, kind="ExternalInput").ap()
    wT = nc.dram_tensor("wT", [DIN, DOUT], _MAIN_DT, kind="ExternalInput").ap()
    G = nc.dram_tensor("G", [P, NPAIR, P], F32, kind="ExternalInput").ap()
    Gt = nc.dram_tensor("Gt", [P, NPAIR, P], F32, kind="ExternalInput").ap()
    eye = nc.dram_tensor("eye", [P, P], F32, kind="ExternalInput").ap()
    bias_rep = nc.dram_tensor("bias_rep", [P, DOUT], F32, kind="ExternalInput").ap()
    y = nc.dram_tensor("y", [TPC, DOUT], F32, kind="ExternalOutput").ap()

    with tile.TileContext(nc) as tc:
        _emit(nc, tc, xT, wT, G, Gt, eye, bias_rep, y)
    nc.compile()
    _CACHE[key] = nc
    return nc


def _maybe_enable_trace():
    """Inject the NTFF profile hook so run_bass_kernel_spmd(trace=True) works
    under axon in this container.  Only used by the dev harness."""
    import types
    try:
        import antenv
        from trn_agent_boot.trn_boot import _ntff_profile_via_ctypes
        import concourse.bass_utils as bass_utils
        hook = _ntff_profile_via_ctypes("/opt/axon/libaxon_pjrt.so")
        mod = types.ModuleType("antenv.axon_hooks")
        mod.get_axon_ntff_profile_hook = lambda: hook
        mod.set_axon_ntff_profile_hook = lambda h: None
        sys.modules["antenv.axon_hooks"] = mod
        antenv.axon_hooks = mod
        bass_utils.upload_artifacts = lambda tmpdir: "local://" + tmpdir
        return True
    except Exception:
        return False


LAST_RESULT = None


def kernel(x, oft_r, W, b):
    global LAST_RESULT
    x = np.ascontiguousarray(np.asarray(x, dtype=np.float32))
    oft_r = np.asarray(oft_r, dtype=np.float32)
    W = np.asarray(W, dtype=np.float32)
    b = np.asarray(b, dtype=np.float32)

    nc = _build()

    # Host-side layout only (no arithmetic): shard/transpose/pad/replicate.
    xf = x.reshape(TOK, DIN)
    wT = np.ascontiguousarray(W.T)
    G = np.zeros((P, NPAIR, P), np.float32)
    Gt = np.zeros((P, NPAIR, P), np.float32)
    oft_t = oft_r.transpose(0, 2, 1)
    for p in range(NPAIR):
        G[:BS, p, :BS] = oft_r[2 * p]
        G[BS:, p, BS:] = oft_r[2 * p + 1]
        Gt[:BS, p, :BS] = oft_t[2 * p]
        Gt[BS:, p, BS:] = oft_t[2 * p + 1]
    eye = np.eye(P, dtype=np.float32)
    bias_rep = np.ascontiguousarray(np.broadcast_to(b, (P, DOUT)))

    shared = {"wT": wT, "G": G, "Gt": Gt, "eye": eye, "bias_rep": bias_rep}
    in_maps = []
    for c in range(N_CORES):
        xTc = np.ascontiguousarray(xf[c * TPC:(c + 1) * TPC].T)
        in_maps.append({"xT": xTc, **shared})

    trace = os.environ.get("KERNEL_TRACE", "0") == "1" and _maybe_enable_trace()
    res = run_bass_kernel_spmd(
        nc, in_maps, core_ids=list(range(N_CORES)), trace=trace,
        trace_cores=[0] if trace else None,
    )
    LAST_RESULT = res

    y = np.concatenate([res.results[c]["y"] for c in range(N_CORES)], axis=0)
    return np.ascontiguousarray(y.reshape(BATCH, SEQ, DOUT))



# revision 16
# speedup vs baseline: 1.3455x; 1.3455x over previous
"""Trainium2 Bass kernel for nn_GPTQOFTLinear.

y = (x rotated by block-diagonal Cayley(oft_r)) @ W^T + b

Strategy (8 NeuronCores, no collectives):
  - Data-parallel shard x over the 8192 tokens (1024 tokens/core); W, oft_r, b
    replicated.
  - On each core:
      1. Cayley transform packed as 32 block-diagonal 128x128 pairs:
         Q = F (I - C)^{-1} with S = skew(data), C = S@S, F = (I-S)^2
           = I - 2S + C.  The inverse is approximated by the 2-term Horner
         series (I - C)^{-1} ~= I + C + C^2 (|C|~0.05 so the truncation
         error ~|C|^3 ~ 1e-4).  3 matmuls/pair, fp16 operands, emitted in
         batched phases so the PE stream stays dense:
            psC = s2^T s2 = -4C          (s2 = 2S)
            psX = C @ (I + C)
            psQ = (C - 2S) @ X,   Q = X + psQ   (X = I + C + C^2)
      2. Rotate: x_rot^T[:, j] = Q_j^T-free matmuls (fp32r), result copied
         into a bf16 SBUF-resident x_rot^T [128, 32, 1024].
      3. Main matmul in bf16: y[t, o] = sum_j x_rot^T[j, t] * W^T[j, o] + b.
         W^T arrives per output-group as a 32 KiB/partition bf16 SBUF tile
         via a single gpsimd *casting* DMA (f32 HBM -> bf16 SBUF), so the
         inner loop has no DMA waits; 8 PSUM banks accumulate 8 token tiles.
  - Host side does only layout (shard/transpose/zero-pad/replicate), no math.
"""

import os
import sys

for _p in ("/opt/trn_rl_repo",):
    if _p not in sys.path and os.path.isdir(_p):
        sys.path.append(_p)

import numpy as np

import concourse.bass as bass  # noqa: E402
import concourse.mybir as mybir  # noqa: E402
import concourse.tile as tile  # noqa: E402
from concourse import bacc  # noqa: E402
from concourse.bass_utils import run_bass_kernel_spmd  # noqa: E402

# Problem shapes (hardcoded per contract).
BATCH, SEQ = 2, 4096
DIN = 4096
DOUT = 4096
BS = 64                      # oft block size
RANK = DIN // BS             # 64 blocks
N_CORES = 8
TOK = BATCH * SEQ            # 8192 tokens
TPC = TOK // N_CORES         # 1024 tokens per core
P = 128
JT = DIN // P                # 32 contraction tiles
NPAIR = RANK // 2            # 32 block pairs
NT = TPC // P                # 8 token tiles per core
OGW = 512                    # output-feature group width
OG = DOUT // OGW             # 8 output groups
XCH = 2                      # x^T j-tiles per staged DMA chunk
JH = JT // 2                 # j-tiles per W og-half tile
ALU = mybir.AluOpType

F32 = mybir.dt.float32
F32R = mybir.dt.float32r
F16 = mybir.dt.float16
BF16 = mybir.dt.bfloat16

# bf16: W tiles og-resident in SBUF via casting DMA, xrot bf16.
# f32r: baseline-style streamed f32r W tiles, xrot f32r.
_MODE = os.environ.get("KERNEL_MODE", "bf16")

_CACHE: dict = {}


def _emit_cayley(nc, tc, g_all, gt_all, eye_sb, Q):
    """Q[:, p, :] = Cayley(pair p), batched phases, fp16 operands.

    g_all/gt_all are the densely packed [P, NPAIR, BS] f32 tiles: partition
    quadrant 0:64 holds block 2p, 64:128 holds block 2p+1."""
    from contextlib import ExitStack

    with ExitStack() as ctx:
        arr = ctx.enter_context(tc.tile_pool(name="cayarr", bufs=1))
        cps = ctx.enter_context(tc.tile_pool(name="cps", bufs=4, space="PSUM"))
        xps = ctx.enter_context(tc.tile_pool(name="xps", bufs=2, space="PSUM"))
        qps = ctx.enter_context(tc.tile_pool(name="qps", bufs=2, space="PSUM"))

        s2 = arr.tile([P, NPAIR, P], F16, name="s2", tag="s2")
        C = arr.tile([P, NPAIR, P], F16, name="C", tag="C")
        X = arr.tile([P, NPAIR, P], F16, name="X", tag="X")

        def veng(i):
            return nc.vector if i % 2 == 0 else nc.gpsimd

        # phase 1: s2 = g - gt (= 2S), block-diagonal; off-diagonal quadrants
        # zeroed once up front.
        nc.vector.memset(s2[:, :, :], 0)
        for p in range(NPAIR):
            veng(p).tensor_sub(
                s2[:BS, p, :BS], g_all[:BS, p, :], gt_all[:BS, p, :])
            veng(p + 1).tensor_sub(
                s2[BS:, p, BS:], g_all[BS:, p, :], gt_all[BS:, p, :])

        # NOTE: gpsimd (Pool) cannot access PSUM on TRN2; PSUM-reading ops go
        # on DVE (tensor_tensor) or Activation (copy/scale only).

        # phase 2: psC = s2^T @ s2 = -4C ; C = -0.25 psC  (Act scale-copy)
        pcs = []
        for p in range(NPAIR):
            ps = cps.tile([P, P], F32, name="cps", tag="cps")
            nc.tensor.matmul(ps, s2[:, p, :], s2[:, p, :])
            pcs.append(ps)
        for p in range(NPAIR):
            nc.scalar.mul(C[:, p, :], pcs[p], -0.25)

        # phase 3: psX = C^T @ C = C^2 ; X = psX + C (DVE), X += I (gpsimd)
        pxs = []
        for p in range(NPAIR):
            ps = xps.tile([P, P], F32, name="xps", tag="xps")
            nc.tensor.matmul(ps, C[:, p, :], C[:, p, :])
            pxs.append(ps)
        for p in range(NPAIR):
            nc.vector.tensor_add(X[:, p, :], pxs[p], C[:, p, :])
            nc.gpsimd.tensor_add(X[:, p, :], X[:, p, :], eye_sb)

        # phase 4: psQ = (C - 2S) @ X via two accumulating matmuls;
        #          Q = X + psQ = F @ X with F = I - 2S + C.
        for p in range(NPAIR):
            ps = qps.tile([P, P], F32, name="qps", tag="qps")
            nc.tensor.matmul(ps, C[:, p, :], X[:, p, :], start=True, stop=False)
            nc.tensor.matmul(ps, s2[:, p, :], X[:, p, :], start=False, stop=True)
            nc.vector.tensor_add(Q[:, p, :], ps, X[:, p, :])


def _emit(nc, tc, xTr, wTr, G, Gt, eye, bias_rep, y):
    """Emit the whole per-core program under TileContext tc."""
    from contextlib import ExitStack

    xrot_dt = BF16 if _MODE == "bf16" else F32R

    ctx = ExitStack()
    with ctx:
        # ---- persistent pools (allocated first, stable addresses) ----
        const = ctx.enter_context(tc.tile_pool(name="const", bufs=1))
        qpool = ctx.enter_context(tc.tile_pool(name="qpool", bufs=1))
        xrot_pool = ctx.enter_context(tc.tile_pool(name="xrotp", bufs=1))
        if _MODE == "bf16":
            wt_pool = ctx.enter_context(tc.tile_pool(name="wtp", bufs=3))
        else:
            wt_pool = ctx.enter_context(tc.tile_pool(name="wtp", bufs=6))
        out_pool = ctx.enter_context(tc.tile_pool(name="outp", bufs=4))
        bias_pool = ctx.enter_context(tc.tile_pool(name="biasp", bufs=2))
        xstage_pool = ctx.enter_context(tc.tile_pool(name="xstagep", bufs=3))

        cayio_pool = ctx.enter_context(tc.tile_pool(name="cayio", bufs=1))

        # DMA priority order on the sync queue: eye + Cayley inputs first
        # (tiny, gate the whole pipeline), then x chunks, then per-og bias.
        eye_sb = const.tile([P, P], F32, name="eye_sb", tag="eye")
        nc.sync.dma_start(out=eye_sb, in_=eye)
        g_all = cayio_pool.tile([P, NPAIR, BS], F32, name="g_all", tag="g_all")
        nc.sync.dma_start(out=g_all, in_=G)
        gt_all = cayio_pool.tile([P, NPAIR, BS], F32, name="gt_all",
                                 tag="gt_all")
        nc.sync.dma_start(out=gt_all, in_=Gt)

        Q = qpool.tile([P, NPAIR, P], F32R, name="Q", tag="Q")
        xrot = xrot_pool.tile([P, JT, TPC], xrot_dt, name="xrot", tag="xrot")

        # x^T staged chunks: issue all DMAs up front (sync HWDGE), consumed
        # in order by the rotation loop.
        xs_tiles = []
        for c in range(JT // XCH):
            xs = xstage_pool.tile([P, XCH, TPC], F32R, name="xs", tag="xs")
            nc.sync.dma_start(out=xs, in_=xTr[:, c * XCH:(c + 1) * XCH, :])
            xs_tiles.append(xs)

        # W og-half-tile prefetch (bf16 mode): gpsimd casting DMA f32 -> bf16.
        # Halves (16 j-tiles each) give finer prefetch granularity at 3 bufs.
        # Issued *after* the Cayley gpsimd work so the W traffic doesn't
        # starve the latency-critical g/gt/x transfers.
        wt_tiles = {}

        def issue_wt(og, half):
            if _MODE != "bf16" or og >= OG:
                return
            wt = wt_pool.tile([P, JH, OGW], BF16, name="wt", tag="wt")
            nc.gpsimd.dma_start(
                out=wt,
                in_=wTr[:, half * JH:(half + 1) * JH,
                        og * OGW:(og + 1) * OGW])
            wt_tiles[(og, half)] = wt

        # ---- Cayley (scoped; PSUM/SBUF freed before rotation/main) ----
        _emit_cayley(nc, tc, g_all, gt_all, eye_sb, Q)

        issue_wt(0, 0)
        issue_wt(0, 1)
        issue_wt(1, 0)

        # ---- rotation: x_rot^T[:, j, :] = Q_j^T.T @ x^T[j-tile] ----
        # (gpsimd cannot read PSUM: copies alternate Act / DVE)
        with tc.tile_pool(name="rpsum", bufs=4, space="PSUM") as rpsum:
            for j in range(JT):
                xs = xs_tiles[j // XCH]
                for th in range(TPC // 512):
                    rps = rpsum.tile([P, 512], F32, name="rps", tag="rps")
                    nc.tensor.matmul(
                        rps,
                        Q[:, j, :],
                        xs[:, j % XCH, th * 512:(th + 1) * 512],
                    )
                    dst = xrot[:, j, th * 512:(th + 1) * 512]
                    if (2 * j + th) % 2 == 0:
                        nc.scalar.copy(out=dst, in_=rps)
                    else:
                        nc.vector.tensor_copy(out=dst, in_=rps)

        # ---- main matmul (all 8 PSUM banks) ----
        with tc.tile_pool(name="mpsum", bufs=1, space="PSUM") as mpsum:
            for og in range(OG):
                bias_og = bias_pool.tile([P, OGW], F32, name="bias_og",
                                         tag="bias_og")
                nc.sync.dma_start(
                    out=bias_og, in_=bias_rep[:, og * OGW:(og + 1) * OGW])

                psums = [
                    mpsum.tile([P, OGW], F32, name=f"mps{tt}", tag=f"mps{tt}")
                    for tt in range(NT)
                ]
                if _MODE == "bf16":
                    for half in range(2):
                        nxt_og, nxt_half = og + (half + 1) // 2, (half + 1) % 2
                        issue_wt(nxt_og + 1, nxt_half)
                        wt = wt_tiles[(og, half)]
                        for jj in range(JH):
                            j = half * JH + jj
                            for tt in range(NT):
                                nc.tensor.matmul(
                                    psums[tt],
                                    xrot[:, j, tt * P:(tt + 1) * P],
                                    wt[:, jj, :],
                                    start=(j == 0),
                                    stop=(j == JT - 1),
                                )
                else:
                    for j in range(JT):
                        wt = wt_pool.tile([P, OGW], F32R, name="wt", tag="wt")
                        nc.sync.dma_start(
                            out=wt,
                            in_=wTr[:, j, og * OGW:(og + 1) * OGW])
                        for tt in range(NT):
                            nc.tensor.matmul(
                                psums[tt],
                                xrot[:, j, tt * P:(tt + 1) * P],
                                wt[:],
                                start=(j == 0),
                                stop=(j == JT - 1),
                            )
                # Drains: DVE adds bias directly from PSUM; odd tiles go
                # Act copy (PSUM->SBUF) + gpsimd bias add (SBUF only).
                for tt in range(NT):
                    out_sb = out_pool.tile([P, OGW], F32, name="out_sb",
                                           tag="out_sb")
                    if tt % 2 == 0:
                        nc.vector.tensor_add(out_sb, psums[tt], bias_og)
                    else:
                        nc.scalar.copy(out=out_sb, in_=psums[tt])
                        nc.gpsimd.tensor_add(out_sb, out_sb, bias_og)
                    nc.sync.dma_start(
                        out=y[tt * P:(tt + 1) * P, og * OGW:(og + 1) * OGW],
                        in_=out_sb)


def _build():
    key = _MODE
    if key in _CACHE:
        return _CACHE[key]
    nc = bacc.Bacc("TRN2", target_bir_lowering=False, debug=False,
                   num_devices=N_CORES)
    xTr = nc.dram_tensor("xTr", [P, JT, TPC], F32R, kind="ExternalInput").ap()
    wTr = nc.dram_tensor("wTr", [P, JT, DOUT], F32, kind="ExternalInput").ap()
    G = nc.dram_tensor("G", [P, NPAIR, BS], F32, kind="ExternalInput").ap()
    Gt = nc.dram_tensor("Gt", [P, NPAIR, BS], F32, kind="ExternalInput").ap()
    eye = nc.dram_tensor("eye", [P, P], F32, kind="ExternalInput").ap()
    bias_rep = nc.dram_tensor("bias_rep", [P, DOUT], F32,
                              kind="ExternalInput").ap()
    y = nc.dram_tensor("y", [TPC, DOUT], F32, kind="ExternalOutput").ap()

    with tile.TileContext(nc) as tc:
        _emit(nc, tc, xTr, wTr, G, Gt, eye, bias_rep, y)
    nc.compile()
    _CACHE[key] = nc
    return nc


def _maybe_enable_trace():
    """Inject the NTFF profile hook so run_bass_kernel_spmd(trace=True) works
    under axon in this container.  Only used by the dev harness."""
    import types
    try:
        import antenv
        from trn_agent_boot.trn_boot import _ntff_profile_via_ctypes
        import concourse.bass_utils as bass_utils
        hook = _ntff_profile_via_ctypes("/opt/axon/libaxon_pjrt.so")
        mod = types.ModuleType("antenv.axon_hooks")
        mod.get_axon_ntff_profile_hook = lambda: hook
        mod.set_axon_ntff_profile_hook = lambda h: None
        sys.modules["antenv.axon_hooks"] = mod
        antenv.axon_hooks = mod
        bass_utils.upload_artifacts = lambda tmpdir: "local://" + tmpdir
        return True
    except Exception:
        return False


LAST_RESULT = None


def kernel(x, oft_r, W, b):
    global LAST_RESULT
    x = np.ascontiguousarray(np.asarray(x, dtype=np.float32))
    oft_r = np.asarray(oft_r, dtype=np.float32)
    W = np.asarray(W, dtype=np.float32)
    b = np.asarray(b, dtype=np.float32)

    nc = _build()

    # Host-side layout only (no arithmetic): shard/transpose/pad/replicate.
    xf = x.reshape(TOK, DIN)
    wTr = np.ascontiguousarray(
        W.T.reshape(JT, P, DOUT).transpose(1, 0, 2))
    # Dense block packing: partitions 0:64 hold block 2p, 64:128 block 2p+1.
    G = np.zeros((P, NPAIR, BS), np.float32)
    Gt = np.zeros((P, NPAIR, BS), np.float32)
    oft_t = oft_r.transpose(0, 2, 1)
    for p in range(NPAIR):
        G[:BS, p, :] = oft_r[2 * p]
        G[BS:, p, :] = oft_r[2 * p + 1]
        Gt[:BS, p, :] = oft_t[2 * p]
        Gt[BS:, p, :] = oft_t[2 * p + 1]
    eye = np.eye(P, dtype=np.float32)
    bias_rep = np.ascontiguousarray(np.broadcast_to(b, (P, DOUT)))

    shared = {"wTr": wTr, "G": G, "Gt": Gt, "eye": eye, "bias_rep": bias_rep}
    in_maps = []
    for c in range(N_CORES):
        xTc = np.ascontiguousarray(
            xf[c * TPC:(c + 1) * TPC].T.reshape(JT, P, TPC).transpose(1, 0, 2))
        in_maps.append({"xTr": xTc, **shared})

    trace = os.environ.get("KERNEL_TRACE", "0") == "1" and _maybe_enable_trace()
    res = run_bass_kernel_spmd(
        nc, in_maps, core_ids=list(range(N_CORES)), trace=trace,
        trace_cores=[0] if trace else None,
    )
    LAST_RESULT = res

    y = np.concatenate([res.results[c]["y"] for c in range(N_CORES)], axis=0)
    return np.ascontiguousarray(y.reshape(BATCH, SEQ, DOUT))


# revision 20
# speedup vs baseline: 1.4902x; 1.1075x over previous
"""Trainium2 Bass kernel for nn_GPTQOFTLinear.

y = (x rotated by block-diagonal Cayley(oft_r)) @ W^T + b

Strategy (8 NeuronCores, no collectives):
  - Data-parallel shard x over the 8192 tokens (1024 tokens/core); W, oft_r, b
    replicated.
  - On each core:
      1. Cayley transform packed as 32 block-diagonal 128x128 pairs:
         Q = F (I - C)^{-1} with S = skew(data), C = S@S, F = (I-S)^2
           = I - 2S + C.  The inverse is approximated by the 2-term Horner
         series (I - C)^{-1} ~= I + C + C^2 (|C|~0.05 so the truncation
         error ~|C|^3 ~ 1e-4).  3 matmuls/pair, fp16 operands, emitted in
         batched phases so the PE stream stays dense:
            psC = s2^T s2 = -4C          (s2 = 2S)
            psX = C @ (I + C)
            psQ = (C - 2S) @ X,   Q = X + psQ   (X = I + C + C^2)
      2. Rotate: x_rot^T[:, j] = Q_j^T-free matmuls (fp32r), result copied
         into a bf16 SBUF-resident x_rot^T [128, 32, 1024].
      3. Main matmul in bf16: y[t, o] = sum_j x_rot^T[j, t] * W^T[j, o] + b.
         W^T arrives per output-group as a 32 KiB/partition bf16 SBUF tile
         via a single gpsimd *casting* DMA (f32 HBM -> bf16 SBUF), so the
         inner loop has no DMA waits; 8 PSUM banks accumulate 8 token tiles.
  - Host side does only layout (shard/transpose/zero-pad/replicate), no math.
"""

import os
import sys

for _p in ("/opt/trn_rl_repo",):
    if _p not in sys.path and os.path.isdir(_p):
        sys.path.append(_p)

import numpy as np

import concourse.bass as bass  # noqa: E402
import concourse.mybir as mybir  # noqa: E402
import concourse.tile as tile  # noqa: E402
from concourse import bacc  # noqa: E402
from concourse.bass_utils import run_bass_kernel_spmd  # noqa: E402

# Problem shapes (hardcoded per contract).
BATCH, SEQ = 2, 4096
DIN = 4096
DOUT = 4096
BS = 64                      # oft block size
RANK = DIN // BS             # 64 blocks
N_CORES = 8
TOK = BATCH * SEQ            # 8192 tokens
TPC = TOK // N_CORES         # 1024 tokens per core
P = 128
JT = DIN // P                # 32 contraction tiles
NPAIR = RANK // 2            # 32 block pairs
NT = TPC // P                # 8 token tiles per core
OGW = 512                    # output-feature group width
OG = DOUT // OGW             # 8 output groups
XCH = 2                      # x^T j-tiles per staged DMA chunk
JH = JT // 2                 # j-tiles per W og-half tile
ALU = mybir.AluOpType

F32 = mybir.dt.float32
F32R = mybir.dt.float32r
F16 = mybir.dt.float16
BF16 = mybir.dt.bfloat16

# bf16: W tiles og-resident in SBUF via casting DMA, xrot bf16.
# f32r: baseline-style streamed f32r W tiles, xrot f32r.
_MODE = os.environ.get("KERNEL_MODE", "bf16")

_CACHE: dict = {}


def _emit_cayley(nc, tc, g_all, gt_all, eye_sb, Q):
    """Q[:, p, :] = Cayley(pair p), batched phases, fp16 operands.

    g_all/gt_all are the densely packed [P, NPAIR, BS] f32 tiles: partition
    quadrant 0:64 holds block 2p, 64:128 holds block 2p+1."""
    from contextlib import ExitStack

    with ExitStack() as ctx:
        arr = ctx.enter_context(tc.tile_pool(name="cayarr", bufs=1))
        cps = ctx.enter_context(tc.tile_pool(name="cps", bufs=4, space="PSUM"))
        xps = ctx.enter_context(tc.tile_pool(name="xps", bufs=2, space="PSUM"))
        qps = ctx.enter_context(tc.tile_pool(name="qps", bufs=2, space="PSUM"))

        s2 = arr.tile([P, NPAIR, P], F16, name="s2", tag="s2")
        C = arr.tile([P, NPAIR, P], F16, name="C", tag="C")
        X = arr.tile([P, NPAIR, P], F16, name="X", tag="X")

        def veng(i):
            return nc.vector if i % 2 == 0 else nc.gpsimd

        # phase 1: s2 = g - gt (= 2S), block-diagonal; off-diagonal quadrants
        # zeroed up front (split across engines; runs while g/gt DMA lands).
        nc.vector.memset(s2[:BS, :, BS:], 0)
        nc.gpsimd.memset(s2[BS:, :, :BS], 0)
        for p in range(NPAIR):
            veng(p).tensor_sub(
                s2[:BS, p, :BS], g_all[:BS, p, :], gt_all[:BS, p, :])
            veng(p + 1).tensor_sub(
                s2[BS:, p, BS:], g_all[BS:, p, :], gt_all[BS:, p, :])

        # NOTE: gpsimd (Pool) cannot access PSUM on TRN2; PSUM-reading ops go
        # on DVE (tensor_tensor) or Activation (copy/scale only).

        # phase 2: psC = s2^T @ s2 = -4C ; C = -0.25 psC  (Act scale-copy)
        pcs = []
        for p in range(NPAIR):
            ps = cps.tile([P, P], F32, name="cps", tag="cps")
            nc.tensor.matmul(ps, s2[:, p, :], s2[:, p, :])
            pcs.append(ps)
        for p in range(NPAIR):
            nc.scalar.mul(C[:, p, :], pcs[p], -0.25)

        # phase 3: psX = C^T @ C = C^2 ; X = psX + C (DVE), X += I (gpsimd)
        pxs = []
        for p in range(NPAIR):
            ps = xps.tile([P, P], F32, name="xps", tag="xps")
            nc.tensor.matmul(ps, C[:, p, :], C[:, p, :])
            pxs.append(ps)
        for p in range(NPAIR):
            nc.vector.tensor_add(X[:, p, :], pxs[p], C[:, p, :])
            nc.gpsimd.tensor_add(X[:, p, :], X[:, p, :], eye_sb)

        # phase 4: psQ = (C - 2S) @ X via two accumulating matmuls;
        #          Q = X + psQ = F @ X with F = I - 2S + C.
        for p in range(NPAIR):
            ps = qps.tile([P, P], F32, name="qps", tag="qps")
            nc.tensor.matmul(ps, C[:, p, :], X[:, p, :], start=True, stop=False)
            nc.tensor.matmul(ps, s2[:, p, :], X[:, p, :], start=False, stop=True)
            nc.vector.tensor_add(Q[:, p, :], ps, X[:, p, :])


def _emit(nc, tc, xTr, wTr, G, Gt, eye, bias_rep, y):
    """Emit the whole per-core program under TileContext tc."""
    from contextlib import ExitStack

    xrot_dt = BF16 if _MODE == "bf16" else F32R

    ctx = ExitStack()
    with ctx:
        # ---- persistent pools (allocated first, stable addresses) ----
        const = ctx.enter_context(tc.tile_pool(name="const", bufs=1))
        qpool = ctx.enter_context(tc.tile_pool(name="qpool", bufs=1))
        xrot_pool = ctx.enter_context(tc.tile_pool(name="xrotp", bufs=1))
        if _MODE == "bf16":
            wt_pool = ctx.enter_context(tc.tile_pool(name="wtp", bufs=3))
        else:
            wt_pool = ctx.enter_context(tc.tile_pool(name="wtp", bufs=6))
        out_pool = ctx.enter_context(tc.tile_pool(name="outp", bufs=4))
        bias_pool = ctx.enter_context(tc.tile_pool(name="biasp", bufs=2))
        xstage_pool = ctx.enter_context(tc.tile_pool(name="xstagep", bufs=3))

        cayio_pool = ctx.enter_context(tc.tile_pool(name="cayio", bufs=1))

        # DMA priority order on the sync queue: eye + Cayley inputs first
        # (tiny, gate the whole pipeline), then x chunks, then per-og bias.
        eye_sb = const.tile([P, P], F32, name="eye_sb", tag="eye")
        nc.sync.dma_start(out=eye_sb, in_=eye)
        g_all = cayio_pool.tile([P, NPAIR, BS], F32, name="g_all", tag="g_all")
        nc.sync.dma_start(out=g_all, in_=G)
        gt_all = cayio_pool.tile([P, NPAIR, BS], F32, name="gt_all",
                                 tag="gt_all")
        nc.sync.dma_start(out=gt_all, in_=Gt)

        Q = qpool.tile([P, NPAIR, P], F32R, name="Q", tag="Q")
        xrot = xrot_pool.tile([P, JT, TPC], xrot_dt, name="xrot", tag="xrot")

        # x^T staged chunks: issue all DMAs up front (sync HWDGE), consumed
        # in order by the rotation loop.
        xs_tiles = []
        for c in range(JT // XCH):
            xs = xstage_pool.tile([P, XCH, TPC], F32R, name="xs", tag="xs")
            nc.sync.dma_start(out=xs, in_=xTr[:, c * XCH:(c + 1) * XCH, :])
            xs_tiles.append(xs)

        # W og-half-tile prefetch (bf16 mode): gpsimd casting DMA f32 -> bf16.
        # Halves (16 j-tiles each) give finer prefetch granularity at 3 bufs.
        # Issued *after* the Cayley gpsimd work so the W traffic doesn't
        # starve the latency-critical g/gt/x transfers.
        wt_tiles = {}

        def issue_wt(og, half):
            if _MODE != "bf16" or og >= OG:
                return
            wt = wt_pool.tile([P, JH, OGW], BF16, name="wt", tag="wt")
            nc.gpsimd.dma_start(
                out=wt,
                in_=wTr[:, half * JH:(half + 1) * JH,
                        og * OGW:(og + 1) * OGW])
            wt_tiles[(og, half)] = wt

        # ---- Cayley (scoped; PSUM/SBUF freed before rotation/main) ----
        _emit_cayley(nc, tc, g_all, gt_all, eye_sb, Q)

        # ---- rotation: x_rot^T[:, j, :] = Q_j^T.T @ x^T[j-tile] ----
        # (gpsimd cannot read PSUM: copies alternate Act / DVE)
        # W og0 prefetch starts late in the rotation so it doesn't steal DMA
        # bandwidth from the latency-critical x chunks.
        with tc.tile_pool(name="rpsum", bufs=4, space="PSUM") as rpsum:
            for j in range(JT):
                if j == 20:
                    issue_wt(0, 0)
                elif j == 30:
                    issue_wt(0, 1)
                xs = xs_tiles[j // XCH]
                for th in range(TPC // 512):
                    rps = rpsum.tile([P, 512], F32, name="rps", tag="rps")
                    nc.tensor.matmul(
                        rps,
                        Q[:, j, :],
                        xs[:, j % XCH, th * 512:(th + 1) * 512],
                    )
                    dst = xrot[:, j, th * 512:(th + 1) * 512]
                    if (2 * j + th) % 2 == 0:
                        nc.scalar.copy(out=dst, in_=rps)
                    else:
                        nc.vector.tensor_copy(out=dst, in_=rps)

        # ---- main matmul (all 8 PSUM banks) ----
        with tc.tile_pool(name="mpsum", bufs=1, space="PSUM") as mpsum:
            for og in range(OG):
                bias_og = bias_pool.tile([P, OGW], F32, name="bias_og",
                                         tag="bias_og")
                nc.sync.dma_start(
                    out=bias_og, in_=bias_rep[:, og * OGW:(og + 1) * OGW])

                # Two 4-bank passes per og (token tiles 0-3, then 4-7), on
                # alternating PSUM bank groups: pass k+1 never waits on pass
                # k's drains, so og boundaries cost nothing.
                for ps in range(2):
                    grp = "A" if ps == 0 else "B"
                    psums = [
                        mpsum.tile([P, OGW], F32, name=f"mp{grp}{t4}",
                                   tag=f"mp{grp}{t4}")
                        for t4 in range(4)
                    ]
                    if _MODE == "bf16":
                        for half in range(2):
                            if ps == 0:
                                issue_wt(og + 1, half)
                            wt = wt_tiles[(og, half)]
                            for jj in range(JH):
                                j = half * JH + jj
                                for t4 in range(4):
                                    tt = ps * 4 + t4
                                    nc.tensor.matmul(
                                        psums[t4],
                                        xrot[:, j, tt * P:(tt + 1) * P],
                                        wt[:, jj, :],
                                        start=(j == 0),
                                        stop=(j == JT - 1),
                                    )
                    else:
                        for j in range(JT):
                            wt = wt_pool.tile([P, OGW], F32R, name="wt",
                                              tag="wt")
                            nc.sync.dma_start(
                                out=wt,
                                in_=wTr[:, j, og * OGW:(og + 1) * OGW])
                            for t4 in range(4):
                                tt = ps * 4 + t4
                                nc.tensor.matmul(
                                    psums[t4],
                                    xrot[:, j, tt * P:(tt + 1) * P],
                                    wt[:],
                                    start=(j == 0),
                                    stop=(j == JT - 1),
                                )
                    # Drains: DVE adds bias directly from PSUM; odd tiles go
                    # Act copy (PSUM->SBUF) + gpsimd bias add (SBUF only).
                    for t4 in range(4):
                        tt = ps * 4 + t4
                        out_sb = out_pool.tile([P, OGW], F32, name="out_sb",
                                               tag="out_sb")
                        if t4 % 2 == 0:
                            nc.vector.tensor_add(out_sb, psums[t4], bias_og)
                        else:
                            nc.scalar.copy(out=out_sb, in_=psums[t4])
                            nc.gpsimd.tensor_add(out_sb, out_sb, bias_og)
                        nc.sync.dma_start(
                            out=y[tt * P:(tt + 1) * P,
                                  og * OGW:(og + 1) * OGW],
                            in_=out_sb)


def _build():
    key = _MODE
    if key in _CACHE:
        return _CACHE[key]
    nc = bacc.Bacc("TRN2", target_bir_lowering=False, debug=False,
                   num_devices=N_CORES)
    xTr = nc.dram_tensor("xTr", [P, JT, TPC], F32R, kind="ExternalInput").ap()
    wTr = nc.dram_tensor("wTr", [P, JT, DOUT], F32, kind="ExternalInput").ap()
    G = nc.dram_tensor("G", [P, NPAIR, BS], F32, kind="ExternalInput").ap()
    Gt = nc.dram_tensor("Gt", [P, NPAIR, BS], F32, kind="ExternalInput").ap()
    eye = nc.dram_tensor("eye", [P, P], F32, kind="ExternalInput").ap()
    bias_rep = nc.dram_tensor("bias_rep", [P, DOUT], F32,
                              kind="ExternalInput").ap()
    y = nc.dram_tensor("y", [TPC, DOUT], F32, kind="ExternalOutput").ap()

    with tile.TileContext(nc) as tc:
        _emit(nc, tc, xTr, wTr, G, Gt, eye, bias_rep, y)
    nc.compile()
    _CACHE[key] = nc
    return nc


def _maybe_enable_trace():
    """Inject the NTFF profile hook so run_bass_kernel_spmd(trace=True) works
    under axon in this container.  Only used by the dev harness."""
    import types
    try:
        import antenv
        from trn_agent_boot.trn_boot import _ntff_profile_via_ctypes
        import concourse.bass_utils as bass_utils
        hook = _ntff_profile_via_ctypes("/opt/axon/libaxon_pjrt.so")
        mod = types.ModuleType("antenv.axon_hooks")
        mod.get_axon_ntff_profile_hook = lambda: hook
        mod.set_axon_ntff_profile_hook = lambda h: None
        sys.modules["antenv.axon_hooks"] = mod
        antenv.axon_hooks = mod
        bass_utils.upload_artifacts = lambda tmpdir: "local://" + tmpdir
        return True
    except Exception:
        return False


LAST_RESULT = None


def kernel(x, oft_r, W, b):
    global LAST_RESULT
    x = np.ascontiguousarray(np.asarray(x, dtype=np.float32))
    oft_r = np.asarray(oft_r, dtype=np.float32)
    W = np.asarray(W, dtype=np.float32)
    b = np.asarray(b, dtype=np.float32)

    nc = _build()

    # Host-side layout only (no arithmetic): shard/transpose/pad/replicate.
    xf = x.reshape(TOK, DIN)
    wTr = np.ascontiguousarray(
        W.T.reshape(JT, P, DOUT).transpose(1, 0, 2))
    # Dense block packing: partitions 0:64 hold block 2p, 64:128 block 2p+1.
    G = np.zeros((P, NPAIR, BS), np.float32)
    Gt = np.zeros((P, NPAIR, BS), np.float32)
    oft_t = oft_r.transpose(0, 2, 1)
    for p in range(NPAIR):
        G[:BS, p, :] = oft_r[2 * p]
        G[BS:, p, :] = oft_r[2 * p + 1]
        Gt[:BS, p, :] = oft_t[2 * p]
        Gt[BS:, p, :] = oft_t[2 * p + 1]
    eye = np.eye(P, dtype=np.float32)
    bias_rep = np.ascontiguousarray(np.broadcast_to(b, (P, DOUT)))

    shared = {"wTr": wTr, "G": G, "Gt": Gt, "eye": eye, "bias_rep": bias_rep}
    in_maps = []
    for c in range(N_CORES):
        xTc = np.ascontiguousarray(
            xf[c * TPC:(c + 1) * TPC].T.reshape(JT, P, TPC).transpose(1, 0, 2))
        in_maps.append({"xTr": xTc, **shared})

    trace = os.environ.get("KERNEL_TRACE", "0") == "1" and _maybe_enable_trace()
    res = run_bass_kernel_spmd(
        nc, in_maps, core_ids=list(range(N_CORES)), trace=trace,
        trace_cores=[0] if trace else None,
    )
    LAST_RESULT = res

    y = np.concatenate([res.results[c]["y"] for c in range(N_CORES)], axis=0)
    return np.ascontiguousarray(y.reshape(BATCH, SEQ, DOUT))
